# revision 51
# baseline (speedup 1.0000x reference)
"""Trainium2 Bass kernel for nn_Loss_20993800143146 (loss_fn).

Computes, over 8 NeuronCores (data-parallel over batch / bh):
    mel_loss  = mean(|mels_pred * mask - mels_target|)           (mean over full tensor)
    stop_loss = sum(-5 * log(stop_pred[b, last_idx_b])) / mask.sum()
    dc        = sum(alignments * band[s,t] * bmask[b]) / (H * lengths.sum() * N)
    out       = mel_loss + stop_loss - 1e-4 * dc

Key algebraic facts:
  * band[s,t] = (s >= clip(5t-50,0,160)) & (s < clip(5t+50,0,160)) is zero for
    t >= 42, and within t < 42 only 2975 of the 6720 (s,t) cells are nonzero.
    The host packs EXACTLY the banded elements densely (zero-padded to a
    rectangle), so the device just sums them - no band weights needed.
  * The mel mask multiplies mels_pred only, so it is folded into the host-side
    packing of the pred tile (masked positions packed as 0), leaving a plain
    sum(|p - t|) on device.

Sharding: batch dim (16 -> 2 per core) for mask/stop/mels, bh dim (64 -> 8 per
core) for alignments. Each core reduces its shard to a [128,8] stats tile; the
host folds partitions / cores and applies the constant-denominator arithmetic.

Heavy data in bf16 (mels) / fp8-e4m3 (alignments) / f16 (stop sidecar and
transpose identity); rel-err budget is 2e-2, measured error stays ~1e-5.

Input DMA is phased through ONE dram tensor on the sync path (the shared HW
queue set executes transfers in doorbell order at full aggregate bandwidth):
  phase S2 (first): banded alignments fp8, stop/mask/iota f16 in the
     split-per-b layout (b = p//64, t = 13*(p%64)+j), per-partition b length
     f32, length prefill f32, b-group indicator f32, f16 transpose identity -
     unblocks the stop and dc chains while the mels stream;
  phases S1a / S1b: mel parts (pred*mask | target bf16 each), split
     asymmetrically (640/400 els) - the subtract+|.|-reduce pairs pipeline
     behind the DMA and the smaller last part minimizes the post-DMA tail.

Stop-term selection per b with no serial cp->PE chain: per-partition
masked-iota max mxp is PE-transposed to a row on partition 0 via the
identity, reduced to the two per-b maxes, PE-broadcast back to all
partitions (ones-column matmul), and a single per-partition
is_equal(mxp, own-b max) * cp select writes the stats column.

Stats cols: 0=dc_w, 1=melA_h1, 2=sel_lnp, 3=mask_cnt, 4=len, 6=melA_h2;
5/7 unused. The [128,8] tile goes out raw; the host folds partitions/cores
(ignoring the unused cols) and applies the constant denominators.
"""

import numpy as np
import ml_dtypes

# Problem constants (hardcoded per contract; kernel.py must be self-contained).
H = 4
B = 16
T = 800
NMEL = 80
S = 160
N = 3
BW = 50
K = T // S  # 5
TC = 42  # band[:, t] == 0 for all t >= TC
NCORES = 8

MEL_ROWS = 2 * T            # 1600 (b,t) rows per core
MEL_PAD_ROWS = 1664         # pad to 128 * 13
MG = 13                     # 80-col groups per partition / stop t's per part
MEL_F = MG * NMEL           # 1040 mel elements per partition per tensor
MH = MEL_F // 2             # 520 els per mel half
ALN_PER_PLANE = 2975        # nonzero band cells per (n, bh) plane
ALN_PER_PART = 560          # 16*560 >= 3*2975, zero padded
ALN_HALF = ALN_PER_PART // 2  # 280

# dAll byte offsets -- phase S2
O_ALN = 0                     # 560 fp8
O_STOP = O_ALN + 560          # 560, 13 f16
O_MASK = O_STOP + 2 * MG      # 586
O_IOTA = O_MASK + 2 * MG      # 612
O_LEN = 640                   # f32
O_LPRE = 644                  # f32: lengths at partitions 0/1, 0 elsewhere
O_IND = 648                   # 2 f32 b-group indicator
O_ID = 656                    # 128 f16 identity row
O_S2END = O_ID + 256          # 912
# phases S1a / S1b -- asymmetric split: the smaller second half shortens
# the post-DMA subtract+reduce tail
MH1 = 640                     # els in mel part 1
MH2 = MEL_F - MH1             # 400 els in mel part 2
O_P1 = O_S2END                # mel pred part1, MH1 bf16
O_T1 = O_P1 + 2 * MH1         # 2192
O_P2 = O_T1 + 2 * MH1         # 3472
O_T2 = O_P2 + 2 * MH2         # 4272
W_ALL = O_T2 + 2 * MH2        # 5072

_CACHE = {}


def _band_bool():
    tr = np.arange(TC)
    mn = np.clip(K * tr - BW, 0, S)
    mx = np.clip(K * tr + BW, 0, S)
    rows = np.arange(S)
    return (rows[:, None] >= mn[None, :]) & (rows[:, None] < mx[None, :])


def _build_bass():
    import concourse.bacc as bacc
    import concourse.tile as tile
    import concourse.mybir as mybir
    from contextlib import ExitStack

    f32 = mybir.dt.float32
    f16 = mybir.dt.float16
    bf16 = mybir.dt.bfloat16
    fp8 = mybir.dt.float8e4
    u8 = mybir.dt.uint8
    Alu = mybir.AluOpType
    Act = mybir.ActivationFunctionType
    Ax = mybir.AxisListType

    nc = bacc.Bacc("TRN2", target_bir_lowering=False, debug=False,
                   num_devices=NCORES)

    dAll = nc.dram_tensor("dAll", [128, W_ALL], u8, kind="ExternalInput").ap()
    out = nc.dram_tensor("out", [128, 8], f32, kind="ExternalOutput").ap()

    with tile.TileContext(nc) as tc:
        with ExitStack() as ctx:
            pool = ctx.enter_context(tc.tile_pool(name="main", bufs=1))
            ppool = ctx.enter_context(tc.tile_pool(name="ps", bufs=1,
                                                   space="PSUM"))

            td_t = pool.tile([128, W_ALL], u8, tag="td")

            # Phased DMA triggers, all on the sync sequencer; the shared
            # queue set executes them in doorbell order: the small S2 phase
            # first (unblocks the whole stop/dc chain early), then the two
            # mel halves so the subtract+|.|-reduce pairs pipeline behind
            # the DMA.
            nc.sync.dma_start(td_t[:, 0:O_P2], dAll[:, 0:O_P2])
            nc.sync.dma_start(td_t[:, O_P2:W_ALL], dAll[:, O_P2:W_ALL])

            id_v = td_t[:, O_ID:O_S2END].bitcast(f16)         # [128, 128]
            ind_v = td_t[:, O_IND:O_IND + 8].bitcast(f32)     # [128, 2]
            aln_v = td_t[:, O_ALN:O_STOP].bitcast(fp8)        # [128, 560]
            stop_v = td_t[:, O_STOP:O_MASK].bitcast(f16)      # [128, 13]
            mask_v = td_t[:, O_MASK:O_IOTA].bitcast(f16)
            iota_v = td_t[:, O_IOTA:O_IOTA + 2 * MG].bitcast(f16)
            lenf_v = td_t[:, O_LEN:O_LEN + 4].bitcast(f32)
            lpre_v = td_t[:, O_LPRE:O_LPRE + 4].bitcast(f32)  # [128, 1]
            p1_v = td_t[:, O_P1:O_T1].bitcast(bf16)           # [128, MH1]
            t1_v = td_t[:, O_T1:O_P2].bitcast(bf16)
            p2_v = td_t[:, O_P2:O_T2].bitcast(bf16)
            t2_v = td_t[:, O_T2:W_ALL].bitcast(bf16)

            st_t = pool.tile([128, 8], f32, tag="st")
            stats = st_t[:]
            ones_t = pool.tile([1, 128], f16, tag="ones")
            nc.vector.memset(ones_t[:], 1.0)

            # ---- ACT: Ln for the stop term ----
            lnp_t = pool.tile([128, MG], f32, tag="lnp")
            nc.scalar.activation(lnp_t[:], stop_v, Act.Ln)

            # ---- DVE stop front + dc term (phase S2 data only) ----
            tl_t = pool.tile([128, MG], f32, tag="tl")
            nc.vector.scalar_tensor_tensor(
                tl_t[:], iota_v, 1.0, mask_v, op0=Alu.bypass, op1=Alu.mult)
            mxp_t = pool.tile([128, 1], f32, tag="mxp")
            nc.vector.tensor_reduce(mxp_t[:], tl_t[:], axis=Ax.X, op=Alu.max)
            nc.vector.tensor_reduce(stats[:, 3:4], mask_v, axis=Ax.X,
                                    op=Alu.add)
            nc.vector.tensor_copy(stats[:, 4:5], lpre_v)
            asum_t = pool.tile([128, ALN_HALF], bf16, tag="asum")
            dcs_t = pool.tile([128, 1], f32, tag="dcs")
            nc.vector.scalar_tensor_tensor(
                asum_t[:], aln_v[:, 0:ALN_HALF], 1.0, aln_v[:, ALN_HALF:],
                op0=Alu.bypass, op1=Alu.add, accum_out=dcs_t[:])
            nc.vector.scalar_tensor_tensor(
                stats[:, 0:1], lenf_v, float(T), dcs_t[:],
                op0=Alu.is_le, op1=Alu.mult)
            eq_t = pool.tile([128, MG], f32, tag="eq")
            cp_t = pool.tile([128, 1], f32, tag="cp")
            nc.vector.scalar_tensor_tensor(
                eq_t[:], tl_t[:], mxp_t[:, 0:1], lnp_t[:],
                op0=Alu.is_equal, op1=Alu.mult, accum_out=cp_t[:])

            # ---- stop tail: transpose mxp, per-b max, broadcast back,
            # then a per-partition select (no serial cp->PE chain) ----
            mxp16_t = pool.tile([128, 1], f16, tag="mxp16")
            nc.vector.tensor_copy(mxp16_t[:], mxp_t[:])
            psA = ppool.tile([1, 128], f32, tag="psA")
            nc.tensor.matmul(psA[:], lhsT=mxp16_t[:], rhs=id_v,
                             start=True, stop=True)
            sbA_t = pool.tile([1, 128], f32, tag="sbA")
            nc.vector.tensor_copy(sbA_t[:], psA[:])
            mb_t = pool.tile([1, 2], f32, tag="mb")
            nc.vector.tensor_reduce(
                mb_t[:], sbA_t[:].rearrange("p (b g) -> p b g", g=64),
                axis=Ax.X, op=Alu.max)
            mb16_t = pool.tile([1, 2], f16, tag="mb16")
            nc.vector.tensor_copy(mb16_t[:], mb_t[:])
            bc2 = ppool.tile([128, 2], f32, tag="bc2")
            nc.tensor.matmul(bc2[:], lhsT=ones_t[:], rhs=mb16_t[0:1, :],
                             start=True, stop=True)
            gs2_t = pool.tile([128, 2], f32, tag="gs2")
            gmx_t = pool.tile([128, 1], f32, tag="gmx")
            nc.vector.scalar_tensor_tensor(
                gs2_t[:], bc2[:], 1.0, ind_v,
                op0=Alu.bypass, op1=Alu.mult, accum_out=gmx_t[:])
            nc.vector.scalar_tensor_tensor(
                stats[:, 2:3], mxp_t[:], gmx_t[:, 0:1], cp_t[:],
                op0=Alu.is_equal, op1=Alu.mult)

            # ---- mel term: halves pipelined behind phases S1a / S1b ----
            d_t = pool.tile([128, MEL_F], bf16, tag="d")
            nc.vector.tensor_sub(d_t[:, 0:MH1], p1_v, t1_v)
            nc.vector.tensor_reduce(stats[:, 1:2], d_t[:, 0:MH1], axis=Ax.X,
                                    op=Alu.add, apply_absolute_value=True)
            nc.vector.tensor_sub(d_t[:, MH1:MEL_F], p2_v, t2_v)
            nc.vector.tensor_reduce(stats[:, 6:7], d_t[:, MH1:MEL_F],
                                    axis=Ax.X, op=Alu.add,
                                    apply_absolute_value=True)

            # ---- stats go out raw; the host folds the 128 partitions ----
            nc.sync.dma_start(out, st_t[:])

    nc.compile()
    return nc


def _get_nc():
    if "nc" not in _CACHE:
        _CACHE["nc"] = _build_bass()
    return _CACHE["nc"]


def make_in_maps(lengths, mask, stop_pred, mels_pred, mels_target, alignments):
    """Shard full inputs into the 8 per-core input dicts."""
    lengths = np.ascontiguousarray(lengths, dtype=np.int32)
    maskf = np.ascontiguousarray(mask).astype(np.float32)
    stop_pred = np.ascontiguousarray(stop_pred, dtype=np.float32)
    mels_pred = np.ascontiguousarray(mels_pred, dtype=np.float32)
    mels_target = np.ascontiguousarray(mels_target, dtype=np.float32)
    alignments = np.ascontiguousarray(alignments, dtype=np.float32)

    bf = ml_dtypes.bfloat16
    f8 = ml_dtypes.float8_e4m3
    band = _band_bool()  # [S, TC]
    el = alignments[:, :, :, :TC][:, :, band]  # [N, B*H, 2975]

    def split13(row, pad):
        o = np.full((64 * MG,), pad, row.dtype)
        o[:T] = row
        return o.reshape(64, MG)

    iota13 = np.concatenate([split13(np.arange(1, T + 1, dtype=np.float16),
                                     np.float16(0))] * 2)  # [128, 13]
    ident = np.eye(128, dtype=np.float16)

    def pad_rows(x2d):
        padded = np.zeros((MEL_PAD_ROWS, NMEL), x2d.dtype)
        padded[:MEL_ROWS] = x2d
        return padded.reshape(128, MEL_F)

    in_maps = []
    for c in range(NCORES):
        bs = slice(2 * c, 2 * c + 2)
        mp = pad_rows((mels_pred[bs] * maskf[bs][..., None])
                      .reshape(MEL_ROWS, NMEL).astype(bf))
        mt = pad_rows(mels_target[bs].reshape(MEL_ROWS, NMEL).astype(bf))

        aln = np.zeros((8, 16 * ALN_PER_PART), f8)
        core_el = el[:, 8 * c:8 * c + 8]          # [3, 8, 2975]
        aln[:, :N * ALN_PER_PLANE] = \
            core_el.transpose(1, 0, 2).reshape(8, N * ALN_PER_PLANE).astype(f8)
        aln = aln.reshape(128, ALN_PER_PART)

        dAll = np.zeros((128, W_ALL), np.uint8)
        dAll[:, O_ID:O_S2END] = ident.view(np.uint8)
        ind2 = np.zeros((128, 2), np.float32)
        ind2[:64, 0] = 1.0
        ind2[64:, 1] = 1.0
        dAll[:, O_IND:O_IND + 8] = ind2.view(np.uint8)
        dAll[:, O_ALN:O_STOP] = aln.view(np.uint8)
        st13 = np.concatenate(
            [split13(stop_pred[2 * c].astype(np.float16), np.float16(1.0)),
             split13(stop_pred[2 * c + 1].astype(np.float16), np.float16(1.0))])
        mk13 = np.concatenate(
            [split13(maskf[2 * c].astype(np.float16), np.float16(0)),
             split13(maskf[2 * c + 1].astype(np.float16), np.float16(0))])
        dAll[:, O_STOP:O_MASK] = st13.view(np.uint8)
        dAll[:, O_MASK:O_IOTA] = mk13.view(np.uint8)
        dAll[:, O_IOTA:O_IOTA + 2 * MG] = iota13.view(np.uint8)
        lenf = np.repeat(lengths[bs].astype(np.float32), 64)  # [128]
        dAll[:, O_LEN:O_LEN + 4] = lenf[:, None].view(np.uint8)
        lpre = np.zeros((128, 1), np.float32)
        lpre[0:2, 0] = lengths[bs]
        dAll[:, O_LPRE:O_LPRE + 4] = lpre.view(np.uint8)
        dAll[:, O_P1:O_T1] = mp[:, 0:MH1].view(np.uint8)
        dAll[:, O_T1:O_P2] = np.ascontiguousarray(mt[:, 0:MH1]).view(np.uint8)
        dAll[:, O_P2:O_T2] = np.ascontiguousarray(mp[:, MH1:]).view(np.uint8)
        dAll[:, O_T2:W_ALL] = np.ascontiguousarray(mt[:, MH1:]).view(np.uint8)

        in_maps.append({"dAll": dAll})
    return in_maps


def combine_partials(partials):
    """partials: list of 8 arrays [128,8] -> final scalar (0-d f32 ndarray).

    Cols 5/7 are unused (uninitialized SBUF) and ignored.
    """
    ps = np.stack([np.asarray(p, dtype=np.float64).reshape(128, 8)
                   for p in partials])
    tot = ps.sum(axis=(0, 1))
    dc_w, mask_cnt, len_sum = tot[0], tot[3], tot[4]
    melA = tot[1] + tot[6]
    sel_lnp = tot[2]
    mel_loss = melA / float(B * T * NMEL)
    stop_loss = -5.0 * sel_lnp / mask_cnt
    dc = dc_w / (H * len_sum * N)
    return np.array(np.float32(mel_loss + stop_loss - 1e-4 * dc))


def kernel(lengths, mask, stop_pred, mels_pred, mels_target, alignments):
    from concourse.bass_utils import run_bass_kernel_spmd

    nc = _get_nc()
    in_maps = make_in_maps(lengths, np.asarray(mask), stop_pred,
                           mels_pred, mels_target, alignments)
    res = run_bass_kernel_spmd(nc, in_maps, list(range(NCORES)))
    return combine_partials([r["out"] for r in res.results])


# revision 52
# speedup vs baseline: 1.0432x; 1.0432x over previous
"""Trainium2 Bass kernel for nn_Loss_20993800143146 (loss_fn).

Computes, over 8 NeuronCores (data-parallel over batch / bh):
    mel_loss  = mean(|mels_pred * mask - mels_target|)           (mean over full tensor)
    stop_loss = sum(-5 * log(stop_pred[b, last_idx_b])) / mask.sum()
    dc        = sum(alignments * band[s,t] * bmask[b]) / (H * lengths.sum() * N)
    out       = mel_loss + stop_loss - 1e-4 * dc

Key algebraic facts:
  * band[s,t] = (s >= clip(5t-50,0,160)) & (s < clip(5t+50,0,160)) is zero for
    t >= 42, and within t < 42 only 2975 of the 6720 (s,t) cells are nonzero.
    The host packs EXACTLY the banded elements densely (zero-padded to a
    rectangle), so the device just sums them - no band weights needed.
  * The mel mask multiplies mels_pred only, so it is folded into the host-side
    packing of the pred tile (masked positions packed as 0), leaving a plain
    sum(|p - t|) on device.

Sharding: batch dim (16 -> 2 per core) for mask/stop/mels, bh dim (64 -> 8 per
core) for alignments. Each core reduces its shard to a [128,8] stats tile; the
host folds partitions / cores and applies the constant-denominator arithmetic.

Heavy data in bf16 (mels) / fp8-e4m3 (alignments) / f16 (stop sidecar and
transpose identity); rel-err budget is 2e-2, measured error stays ~1e-5.

Input DMA is phased through ONE dram tensor on the sync path (the shared HW
queue set executes transfers in doorbell order at full aggregate bandwidth):
  phase S2 (first): banded alignments fp8, stop/mask/iota f16 in the
     split-per-b layout (b = p//64, t = 13*(p%64)+j), per-partition b length
     f32, length prefill f32, b-group indicator f32, f16 transpose identity -
     unblocks the stop and dc chains while the mels stream;
  phases S1a / S1b: mel parts (pred*mask | target bf16 each), split
     asymmetrically (640/400 els) - the subtract+|.|-reduce pairs pipeline
     behind the DMA and the smaller last part minimizes the post-DMA tail.

Stop-term selection per b with no serial cp->PE chain: per-partition
masked-iota max mxp is PE-transposed to a row on partition 0 via the
identity, reduced to the two per-b maxes, PE-broadcast back to all
partitions (ones-column matmul), and a single per-partition
is_equal(mxp, own-b max) * cp select writes the stats column.

Stats cols: 0=dc_w, 1=melA_h1, 2=sel_lnp, 3=mask_cnt, 4=len, 6=melA_h2;
5/7 unused. The [128,8] tile goes out raw; the host folds partitions/cores
(ignoring the unused cols) and applies the constant denominators.
"""

import numpy as np
import ml_dtypes

# Problem constants (hardcoded per contract; kernel.py must be self-contained).
H = 4
B = 16
T = 800
NMEL = 80
S = 160
N = 3
BW = 50
K = T // S  # 5
TC = 42  # band[:, t] == 0 for all t >= TC
NCORES = 8

MEL_ROWS = 2 * T            # 1600 (b,t) rows per core
MEL_PAD_ROWS = 1664         # pad to 128 * 13
MG = 13                     # 80-col groups per partition / stop t's per part
MEL_F = MG * NMEL           # 1040 mel elements per partition per tensor
MH = MEL_F // 2             # 520 els per mel half
ALN_PER_PLANE = 2975        # nonzero band cells per (n, bh) plane
ALN_PER_PART = 560          # 16*560 >= 3*2975, zero padded
ALN_HALF = ALN_PER_PART // 2  # 280

# dAll byte offsets -- phase S2
O_ALN = 0                     # 560 fp8
O_STOP = O_ALN + 560          # 560, 13 f16
O_MASK = O_STOP + 2 * MG      # 586
O_IOTA = O_MASK + 2 * MG      # 612
O_LEN = 640                   # f32
O_LPRE = 644                  # f32: lengths at partitions 0/1, 0 elsewhere
O_IND = 648                   # 2 f32 b-group indicator
O_ID = 656                    # 128 f16 identity row
O_S2END = O_ID + 256          # 912
# phases S1a / S1b -- asymmetric split: the smaller second half shortens
# the post-DMA subtract+reduce tail
MH1 = 640                     # els in mel part 1
MH2 = MEL_F - MH1             # 400 els in mel part 2
O_P1 = O_S2END                # mel pred part1, MH1 bf16
O_T1 = O_P1 + 2 * MH1         # 2192
O_P2 = O_T1 + 2 * MH1         # 3472
O_T2 = O_P2 + 2 * MH2         # 4272
W_ALL = O_T2 + 2 * MH2        # 5072

_CACHE = {}


def _band_bool():
    tr = np.arange(TC)
    mn = np.clip(K * tr - BW, 0, S)
    mx = np.clip(K * tr + BW, 0, S)
    rows = np.arange(S)
    return (rows[:, None] >= mn[None, :]) & (rows[:, None] < mx[None, :])


def _build_bass():
    import concourse.bacc as bacc
    import concourse.tile as tile
    import concourse.mybir as mybir
    from contextlib import ExitStack

    f32 = mybir.dt.float32
    f16 = mybir.dt.float16
    bf16 = mybir.dt.bfloat16
    fp8 = mybir.dt.float8e4
    u8 = mybir.dt.uint8
    Alu = mybir.AluOpType
    Act = mybir.ActivationFunctionType
    Ax = mybir.AxisListType

    nc = bacc.Bacc("TRN2", target_bir_lowering=False, debug=False,
                   num_devices=NCORES)

    dAll = nc.dram_tensor("dAll", [128, W_ALL], u8, kind="ExternalInput").ap()
    out = nc.dram_tensor("out", [128, 8], f32, kind="ExternalOutput").ap()

    with tile.TileContext(nc) as tc:
        with ExitStack() as ctx:
            pool = ctx.enter_context(tc.tile_pool(name="main", bufs=1))
            ppool = ctx.enter_context(tc.tile_pool(name="ps", bufs=1,
                                                   space="PSUM"))

            td_t = pool.tile([128, W_ALL], u8, tag="td")

            # Phased DMA triggers, all on the sync sequencer; the shared
            # queue set executes them in doorbell order: the small S2 phase
            # first (unblocks the whole stop/dc chain early), then the two
            # mel halves so the subtract+|.|-reduce pairs pipeline behind
            # the DMA.
            nc.sync.dma_start(td_t[:, 0:O_S2END], dAll[:, 0:O_S2END])
            nc.sync.dma_start(td_t[:, O_P1:O_P2], dAll[:, O_P1:O_P2])
            nc.sync.dma_start(td_t[:, O_P2:W_ALL], dAll[:, O_P2:W_ALL])

            id_v = td_t[:, O_ID:O_S2END].bitcast(f16)         # [128, 128]
            ind_v = td_t[:, O_IND:O_IND + 8].bitcast(f32)     # [128, 2]
            aln_v = td_t[:, O_ALN:O_STOP].bitcast(fp8)        # [128, 560]
            stop_v = td_t[:, O_STOP:O_MASK].bitcast(f16)      # [128, 13]
            mask_v = td_t[:, O_MASK:O_IOTA].bitcast(f16)
            iota_v = td_t[:, O_IOTA:O_IOTA + 2 * MG].bitcast(f16)
            lenf_v = td_t[:, O_LEN:O_LEN + 4].bitcast(f32)
            lpre_v = td_t[:, O_LPRE:O_LPRE + 4].bitcast(f32)  # [128, 1]
            p1_v = td_t[:, O_P1:O_T1].bitcast(bf16)           # [128, MH1]
            t1_v = td_t[:, O_T1:O_P2].bitcast(bf16)
            p2_v = td_t[:, O_P2:O_T2].bitcast(bf16)
            t2_v = td_t[:, O_T2:W_ALL].bitcast(bf16)

            st_t = pool.tile([128, 8], f32, tag="st")
            stats = st_t[:]
            ones_t = pool.tile([1, 128], f16, tag="ones")
            nc.vector.memset(ones_t[:], 1.0)

            # ---- ACT: Ln for the stop term ----
            lnp_t = pool.tile([128, MG], f32, tag="lnp")
            nc.scalar.activation(lnp_t[:], stop_v, Act.Ln)

            # ---- DVE stop front + dc term (phase S2 data only) ----
            tl_t = pool.tile([128, MG], f32, tag="tl")
            nc.vector.scalar_tensor_tensor(
                tl_t[:], iota_v, 1.0, mask_v, op0=Alu.bypass, op1=Alu.mult)
            mxp_t = pool.tile([128, 1], f32, tag="mxp")
            nc.vector.tensor_reduce(mxp_t[:], tl_t[:], axis=Ax.X, op=Alu.max)
            nc.vector.tensor_reduce(stats[:, 3:4], mask_v, axis=Ax.X,
                                    op=Alu.add)
            nc.vector.tensor_copy(stats[:, 4:5], lpre_v)
            asum_t = pool.tile([128, ALN_HALF], bf16, tag="asum")
            dcs_t = pool.tile([128, 1], f32, tag="dcs")
            nc.vector.scalar_tensor_tensor(
                asum_t[:], aln_v[:, 0:ALN_HALF], 1.0, aln_v[:, ALN_HALF:],
                op0=Alu.bypass, op1=Alu.add, accum_out=dcs_t[:])
            nc.vector.scalar_tensor_tensor(
                stats[:, 0:1], lenf_v, float(T), dcs_t[:],
                op0=Alu.is_le, op1=Alu.mult)
            eq_t = pool.tile([128, MG], f32, tag="eq")
            cp_t = pool.tile([128, 1], f32, tag="cp")
            nc.vector.scalar_tensor_tensor(
                eq_t[:], tl_t[:], mxp_t[:, 0:1], lnp_t[:],
                op0=Alu.is_equal, op1=Alu.mult, accum_out=cp_t[:])

            # ---- stop tail: transpose mxp, per-b max, broadcast back,
            # then a per-partition select (no serial cp->PE chain) ----
            mxp16_t = pool.tile([128, 1], f16, tag="mxp16")
            nc.vector.tensor_copy(mxp16_t[:], mxp_t[:])
            psA = ppool.tile([1, 128], f32, tag="psA")
            nc.tensor.matmul(psA[:], lhsT=mxp16_t[:], rhs=id_v,
                             start=True, stop=True)
            sbA_t = pool.tile([1, 128], f32, tag="sbA")
            nc.vector.tensor_copy(sbA_t[:], psA[:])
            mb_t = pool.tile([1, 2], f32, tag="mb")
            nc.vector.tensor_reduce(
                mb_t[:], sbA_t[:].rearrange("p (b g) -> p b g", g=64),
                axis=Ax.X, op=Alu.max)
            mb16_t = pool.tile([1, 2], f16, tag="mb16")
            nc.vector.tensor_copy(mb16_t[:], mb_t[:])
            bc2 = ppool.tile([128, 2], f32, tag="bc2")
            nc.tensor.matmul(bc2[:], lhsT=ones_t[:], rhs=mb16_t[0:1, :],
                             start=True, stop=True)
            gs2_t = pool.tile([128, 2], f32, tag="gs2")
            gmx_t = pool.tile([128, 1], f32, tag="gmx")
            nc.vector.scalar_tensor_tensor(
                gs2_t[:], bc2[:], 1.0, ind_v,
                op0=Alu.bypass, op1=Alu.mult, accum_out=gmx_t[:])
            nc.vector.scalar_tensor_tensor(
                stats[:, 2:3], mxp_t[:], gmx_t[:, 0:1], cp_t[:],
                op0=Alu.is_equal, op1=Alu.mult)

            # ---- mel term: halves pipelined behind phases S1a / S1b ----
            d_t = pool.tile([128, MEL_F], bf16, tag="d")
            nc.vector.tensor_sub(d_t[:, 0:MH1], p1_v, t1_v)
            nc.vector.tensor_reduce(stats[:, 1:2], d_t[:, 0:MH1], axis=Ax.X,
                                    op=Alu.add, apply_absolute_value=True)
            nc.vector.tensor_sub(d_t[:, MH1:MEL_F], p2_v, t2_v)
            nc.vector.tensor_reduce(stats[:, 6:7], d_t[:, MH1:MEL_F],
                                    axis=Ax.X, op=Alu.add,
                                    apply_absolute_value=True)

            # ---- stats go out raw; the host folds the 128 partitions ----
            nc.sync.dma_start(out, st_t[:])

    nc.compile()
    return nc


def _get_nc():
    if "nc" not in _CACHE:
        _CACHE["nc"] = _build_bass()
    return _CACHE["nc"]


def make_in_maps(lengths, mask, stop_pred, mels_pred, mels_target, alignments):
    """Shard full inputs into the 8 per-core input dicts."""
    lengths = np.ascontiguousarray(lengths, dtype=np.int32)
    maskf = np.ascontiguousarray(mask).astype(np.float32)
    stop_pred = np.ascontiguousarray(stop_pred, dtype=np.float32)
    mels_pred = np.ascontiguousarray(mels_pred, dtype=np.float32)
    mels_target = np.ascontiguousarray(mels_target, dtype=np.float32)
    alignments = np.ascontiguousarray(alignments, dtype=np.float32)

    bf = ml_dtypes.bfloat16
    f8 = ml_dtypes.float8_e4m3
    band = _band_bool()  # [S, TC]
    el = alignments[:, :, :, :TC][:, :, band]  # [N, B*H, 2975]

    def split13(row, pad):
        o = np.full((64 * MG,), pad, row.dtype)
        o[:T] = row
        return o.reshape(64, MG)

    iota13 = np.concatenate([split13(np.arange(1, T + 1, dtype=np.float16),
                                     np.float16(0))] * 2)  # [128, 13]
    ident = np.eye(128, dtype=np.float16)

    def pad_rows(x2d):
        padded = np.zeros((MEL_PAD_ROWS, NMEL), x2d.dtype)
        padded[:MEL_ROWS] = x2d
        return padded.reshape(128, MEL_F)

    in_maps = []
    for c in range(NCORES):
        bs = slice(2 * c, 2 * c + 2)
        mp = pad_rows((mels_pred[bs] * maskf[bs][..., None])
                      .reshape(MEL_ROWS, NMEL).astype(bf))
        mt = pad_rows(mels_target[bs].reshape(MEL_ROWS, NMEL).astype(bf))

        aln = np.zeros((8, 16 * ALN_PER_PART), f8)
        core_el = el[:, 8 * c:8 * c + 8]          # [3, 8, 2975]
        aln[:, :N * ALN_PER_PLANE] = \
            core_el.transpose(1, 0, 2).reshape(8, N * ALN_PER_PLANE).astype(f8)
        aln = aln.reshape(128, ALN_PER_PART)

        dAll = np.zeros((128, W_ALL), np.uint8)
        dAll[:, O_ID:O_S2END] = ident.view(np.uint8)
        ind2 = np.zeros((128, 2), np.float32)
        ind2[:64, 0] = 1.0
        ind2[64:, 1] = 1.0
        dAll[:, O_IND:O_IND + 8] = ind2.view(np.uint8)
        dAll[:, O_ALN:O_STOP] = aln.view(np.uint8)
        st13 = np.concatenate(
            [split13(stop_pred[2 * c].astype(np.float16), np.float16(1.0)),
             split13(stop_pred[2 * c + 1].astype(np.float16), np.float16(1.0))])
        mk13 = np.concatenate(
            [split13(maskf[2 * c].astype(np.float16), np.float16(0)),
             split13(maskf[2 * c + 1].astype(np.float16), np.float16(0))])
        dAll[:, O_STOP:O_MASK] = st13.view(np.uint8)
        dAll[:, O_MASK:O_IOTA] = mk13.view(np.uint8)
        dAll[:, O_IOTA:O_IOTA + 2 * MG] = iota13.view(np.uint8)
        lenf = np.repeat(lengths[bs].astype(np.float32), 64)  # [128]
        dAll[:, O_LEN:O_LEN + 4] = lenf[:, None].view(np.uint8)
        lpre = np.zeros((128, 1), np.float32)
        lpre[0:2, 0] = lengths[bs]
        dAll[:, O_LPRE:O_LPRE + 4] = lpre.view(np.uint8)
        dAll[:, O_P1:O_T1] = mp[:, 0:MH1].view(np.uint8)
        dAll[:, O_T1:O_P2] = np.ascontiguousarray(mt[:, 0:MH1]).view(np.uint8)
        dAll[:, O_P2:O_T2] = np.ascontiguousarray(mp[:, MH1:]).view(np.uint8)
        dAll[:, O_T2:W_ALL] = np.ascontiguousarray(mt[:, MH1:]).view(np.uint8)

        in_maps.append({"dAll": dAll})
    return in_maps


def combine_partials(partials):
    """partials: list of 8 arrays [128,8] -> final scalar (0-d f32 ndarray).

    Cols 5/7 are unused (uninitialized SBUF) and ignored.
    """
    ps = np.stack([np.asarray(p, dtype=np.float64).reshape(128, 8)
                   for p in partials])
    tot = ps.sum(axis=(0, 1))
    dc_w, mask_cnt, len_sum = tot[0], tot[3], tot[4]
    melA = tot[1] + tot[6]
    sel_lnp = tot[2]
    mel_loss = melA / float(B * T * NMEL)
    stop_loss = -5.0 * sel_lnp / mask_cnt
    dc = dc_w / (H * len_sum * N)
    return np.array(np.float32(mel_loss + stop_loss - 1e-4 * dc))


def kernel(lengths, mask, stop_pred, mels_pred, mels_target, alignments):
    from concourse.bass_utils import run_bass_kernel_spmd

    nc = _get_nc()
    in_maps = make_in_maps(lengths, np.asarray(mask), stop_pred,
                           mels_pred, mels_target, alignments)
    res = run_bass_kernel_spmd(nc, in_maps, list(range(NCORES)))
    return combine_partials([r["out"] for r in res.results])


# revision 53
# speedup vs baseline: 1.0540x; 1.0103x over previous
"""Trainium2 Bass kernel for nn_Loss_20993800143146 (loss_fn).

Computes, over 8 NeuronCores (data-parallel over batch / bh):
    mel_loss  = mean(|mels_pred * mask - mels_target|)           (mean over full tensor)
    stop_loss = sum(-5 * log(stop_pred[b, last_idx_b])) / mask.sum()
    dc        = sum(alignments * band[s,t] * bmask[b]) / (H * lengths.sum() * N)
    out       = mel_loss + stop_loss - 1e-4 * dc

Key algebraic facts:
  * band[s,t] = (s >= clip(5t-50,0,160)) & (s < clip(5t+50,0,160)) is zero for
    t >= 42, and within t < 42 only 2975 of the 6720 (s,t) cells are nonzero.
    The host packs EXACTLY the banded elements densely (zero-padded to a
    rectangle), so the device just sums them - no band weights needed.
  * The mel mask multiplies mels_pred only, so it is folded into the host-side
    packing of the pred tile (masked positions packed as 0), leaving a plain
    sum(|p - t|) on device.

Sharding: batch dim (16 -> 2 per core) for mask/stop/mels, bh dim (64 -> 8 per
core) for alignments. Each core reduces its shard to a [128,8] stats tile; the
host folds partitions / cores and applies the constant-denominator arithmetic.

Heavy data in bf16 (mels) / fp8-e4m3 (alignments) / f16 (stop sidecar and
transpose identity); rel-err budget is 2e-2, measured error stays ~1e-5.

Input DMA is phased through ONE dram tensor on the sync path (the shared HW
queue set executes transfers in doorbell order at full aggregate bandwidth):
  phase S2 (first): banded alignments fp8, stop/mask/iota f16 in the
     split-per-b layout (b = p//64, t = 13*(p%64)+j), per-partition b length
     f32, length prefill f32, b-group indicator f32, f16 transpose identity -
     unblocks the stop and dc chains while the mels stream;
  phases S1a / S1b: mel parts (pred*mask | target bf16 each), split
     asymmetrically (640/400 els) - the subtract+|.|-reduce pairs pipeline
     behind the DMA and the smaller last part minimizes the post-DMA tail.

Stop-term selection per b with no serial cp->PE chain: per-partition
masked-iota max mxp is PE-transposed to a row on partition 0 via the
identity, reduced to the two per-b maxes, PE-broadcast back to all
partitions (ones-column matmul), and a single per-partition
is_equal(mxp, own-b max) * cp select writes the stats column.

Stats cols: 0=dc_w, 1=melA_h1, 2=sel_lnp, 3=mask_cnt, 4=len, 6=melA_h2;
5/7 unused. The [128,8] tile goes out raw; the host folds partitions/cores
(ignoring the unused cols) and applies the constant denominators.
"""

import numpy as np
import ml_dtypes

# Problem constants (hardcoded per contract; kernel.py must be self-contained).
H = 4
B = 16
T = 800
NMEL = 80
S = 160
N = 3
BW = 50
K = T // S  # 5
TC = 42  # band[:, t] == 0 for all t >= TC
NCORES = 8

MEL_ROWS = 2 * T            # 1600 (b,t) rows per core
MEL_PAD_ROWS = 1664         # pad to 128 * 13
MG = 13                     # 80-col groups per partition / stop t's per part
MEL_F = MG * NMEL           # 1040 mel elements per partition per tensor
MH = MEL_F // 2             # 520 els per mel half
ALN_PER_PLANE = 2975        # nonzero band cells per (n, bh) plane
ALN_PER_PART = 560          # 16*560 >= 3*2975, zero padded
ALN_HALF = ALN_PER_PART // 2  # 280

# dAll byte offsets -- phase S2
O_ALN = 0                     # 560 fp8
O_STOP = O_ALN + 560          # 560, 13 f16
O_MASK = O_STOP + 2 * MG      # 586
O_IOTA = O_MASK + 2 * MG      # 612
O_LEN = 640                   # f32
O_LPRE = 644                  # f32: lengths at partitions 0/1, 0 elsewhere
O_IND = 648                   # 2 f32 b-group indicator
O_ID = 656                    # 128 f16 identity row
O_S2END = O_ID + 256          # 912
# phases S1a / S1b -- asymmetric split: the smaller second half shortens
# the post-DMA subtract+reduce tail
MH1 = 640                     # els in mel part 1
MH2 = MEL_F - MH1             # 400 els in mel part 2
O_P1 = O_S2END                # mel pred part1, MH1 bf16
O_T1 = O_P1 + 2 * MH1         # 2192
O_P2 = O_T1 + 2 * MH1         # 3472
O_T2 = O_P2 + 2 * MH2         # 4272
W_ALL = O_T2 + 2 * MH2        # 5072

_CACHE = {}


def _band_bool():
    tr = np.arange(TC)
    mn = np.clip(K * tr - BW, 0, S)
    mx = np.clip(K * tr + BW, 0, S)
    rows = np.arange(S)
    return (rows[:, None] >= mn[None, :]) & (rows[:, None] < mx[None, :])


def _build_bass():
    import concourse.bacc as bacc
    import concourse.tile as tile
    import concourse.mybir as mybir
    from contextlib import ExitStack

    f32 = mybir.dt.float32
    f16 = mybir.dt.float16
    bf16 = mybir.dt.bfloat16
    fp8 = mybir.dt.float8e4
    u8 = mybir.dt.uint8
    Alu = mybir.AluOpType
    Act = mybir.ActivationFunctionType
    Ax = mybir.AxisListType

    nc = bacc.Bacc("TRN2", target_bir_lowering=False, debug=False,
                   num_devices=NCORES)

    dAll = nc.dram_tensor("dAll", [128, W_ALL], u8, kind="ExternalInput").ap()
    out = nc.dram_tensor("out", [128, 8], f32, kind="ExternalOutput").ap()

    with tile.TileContext(nc) as tc:
        with ExitStack() as ctx:
            pool = ctx.enter_context(tc.tile_pool(name="main", bufs=1))
            ppool = ctx.enter_context(tc.tile_pool(name="ps", bufs=1,
                                                   space="PSUM"))

            td_t = pool.tile([128, W_ALL], u8, tag="td")

            # Phased DMA triggers, all on the sync sequencer; the shared
            # queue set executes them in doorbell order: the small S2 phase
            # first (unblocks the whole stop/dc chain early), then the two
            # mel halves so the subtract+|.|-reduce pairs pipeline behind
            # the DMA.
            nc.sync.dma_start(td_t[:, 0:O_S2END], dAll[:, 0:O_S2END])
            nc.sync.dma_start(td_t[:, O_P1:O_P2], dAll[:, O_P1:O_P2])
            nc.sync.dma_start(td_t[:, O_P2:W_ALL], dAll[:, O_P2:W_ALL])

            id_v = td_t[:, O_ID:O_S2END].bitcast(f16)         # [128, 128]
            ind_v = td_t[:, O_IND:O_IND + 8].bitcast(f32)     # [128, 2]
            aln_v = td_t[:, O_ALN:O_STOP].bitcast(fp8)        # [128, 560]
            stop_v = td_t[:, O_STOP:O_MASK].bitcast(f16)      # [128, 13]
            mask_v = td_t[:, O_MASK:O_IOTA].bitcast(f16)
            iota_v = td_t[:, O_IOTA:O_IOTA + 2 * MG].bitcast(f16)
            lenf_v = td_t[:, O_LEN:O_LEN + 4].bitcast(f32)
            lpre_v = td_t[:, O_LPRE:O_LPRE + 4].bitcast(f32)  # [128, 1]
            p1_v = td_t[:, O_P1:O_T1].bitcast(bf16)           # [128, MH1]
            t1_v = td_t[:, O_T1:O_P2].bitcast(bf16)
            p2_v = td_t[:, O_P2:O_T2].bitcast(bf16)
            t2_v = td_t[:, O_T2:W_ALL].bitcast(bf16)

            st_t = pool.tile([128, 8], f32, tag="st")
            stats = st_t[:]
            ones_t = pool.tile([1, 128], f16, tag="ones")
            nc.vector.memset(ones_t[:], 1.0)

            # ---- ACT: Ln for the stop term ----
            lnp_t = pool.tile([128, MG], f32, tag="lnp")
            nc.scalar.activation(lnp_t[:], stop_v, Act.Ln)

            # ---- DVE stop front + dc term (phase S2 data only) ----
            tl_t = pool.tile([128, MG], f32, tag="tl")
            nc.vector.scalar_tensor_tensor(
                tl_t[:], iota_v, 1.0, mask_v, op0=Alu.bypass, op1=Alu.mult)
            mxp_t = pool.tile([128, 1], f32, tag="mxp")
            nc.vector.tensor_reduce(mxp_t[:], tl_t[:], axis=Ax.X, op=Alu.max)
            nc.vector.tensor_reduce(stats[:, 3:4], mask_v, axis=Ax.X,
                                    op=Alu.add)
            nc.vector.tensor_copy(stats[:, 4:5], lpre_v)
            asum_t = pool.tile([128, ALN_HALF], bf16, tag="asum")
            dcs_t = pool.tile([128, 1], f32, tag="dcs")
            nc.vector.scalar_tensor_tensor(
                asum_t[:], aln_v[:, 0:ALN_HALF], 1.0, aln_v[:, ALN_HALF:],
                op0=Alu.bypass, op1=Alu.add, accum_out=dcs_t[:])
            nc.vector.scalar_tensor_tensor(
                stats[:, 0:1], lenf_v, float(T), dcs_t[:],
                op0=Alu.is_le, op1=Alu.mult)
            eq_t = pool.tile([128, MG], f32, tag="eq")
            cp_t = pool.tile([128, 1], f32, tag="cp")
            nc.vector.scalar_tensor_tensor(
                eq_t[:], tl_t[:], mxp_t[:, 0:1], lnp_t[:],
                op0=Alu.is_equal, op1=Alu.mult, accum_out=cp_t[:])

            # ---- stop tail: transpose mxp, per-b max, broadcast back,
            # then a per-partition select (no serial cp->PE chain) ----
            mxp16_t = pool.tile([128, 1], f16, tag="mxp16")
            nc.vector.tensor_copy(mxp16_t[:], mxp_t[:])
            psA = ppool.tile([1, 128], f32, tag="psA")
            nc.tensor.matmul(psA[:], lhsT=mxp16_t[:], rhs=id_v,
                             start=True, stop=True)
            sbA_t = pool.tile([1, 128], f16, tag="sbA")
            nc.vector.tensor_copy(sbA_t[:], psA[:])
            mb_t = pool.tile([1, 2], f16, tag="mb")
            nc.vector.tensor_reduce(
                mb_t[:], sbA_t[:].rearrange("p (b g) -> p b g", g=64),
                axis=Ax.X, op=Alu.max)
            bc2 = ppool.tile([128, 2], f32, tag="bc2")
            nc.tensor.matmul(bc2[:], lhsT=ones_t[:], rhs=mb_t[0:1, :],
                             start=True, stop=True)
            gs2_t = pool.tile([128, 2], f32, tag="gs2")
            gmx_t = pool.tile([128, 1], f32, tag="gmx")
            nc.vector.scalar_tensor_tensor(
                gs2_t[:], bc2[:], 1.0, ind_v,
                op0=Alu.bypass, op1=Alu.mult, accum_out=gmx_t[:])
            nc.vector.scalar_tensor_tensor(
                stats[:, 2:3], mxp_t[:], gmx_t[:, 0:1], cp_t[:],
                op0=Alu.is_equal, op1=Alu.mult)

            # ---- mel term: halves pipelined behind phases S1a / S1b ----
            d_t = pool.tile([128, MEL_F], bf16, tag="d")
            nc.vector.tensor_sub(d_t[:, 0:MH1], p1_v, t1_v)
            nc.vector.tensor_reduce(stats[:, 1:2], d_t[:, 0:MH1], axis=Ax.X,
                                    op=Alu.add, apply_absolute_value=True)
            nc.vector.tensor_sub(d_t[:, MH1:MEL_F], p2_v, t2_v)
            nc.vector.tensor_reduce(stats[:, 6:7], d_t[:, MH1:MEL_F],
                                    axis=Ax.X, op=Alu.add,
                                    apply_absolute_value=True)

            # ---- stats go out raw; the host folds the 128 partitions ----
            nc.sync.dma_start(out, st_t[:])

    nc.compile()
    return nc


def _get_nc():
    if "nc" not in _CACHE:
        _CACHE["nc"] = _build_bass()
    return _CACHE["nc"]


def make_in_maps(lengths, mask, stop_pred, mels_pred, mels_target, alignments):
    """Shard full inputs into the 8 per-core input dicts."""
    lengths = np.ascontiguousarray(lengths, dtype=np.int32)
    maskf = np.ascontiguousarray(mask).astype(np.float32)
    stop_pred = np.ascontiguousarray(stop_pred, dtype=np.float32)
    mels_pred = np.ascontiguousarray(mels_pred, dtype=np.float32)
    mels_target = np.ascontiguousarray(mels_target, dtype=np.float32)
    alignments = np.ascontiguousarray(alignments, dtype=np.float32)

    bf = ml_dtypes.bfloat16
    f8 = ml_dtypes.float8_e4m3
    band = _band_bool()  # [S, TC]
    el = alignments[:, :, :, :TC][:, :, band]  # [N, B*H, 2975]

    def split13(row, pad):
        o = np.full((64 * MG,), pad, row.dtype)
        o[:T] = row
        return o.reshape(64, MG)

    iota13 = np.concatenate([split13(np.arange(1, T + 1, dtype=np.float16),
                                     np.float16(0))] * 2)  # [128, 13]
    ident = np.eye(128, dtype=np.float16)

    def pad_rows(x2d):
        padded = np.zeros((MEL_PAD_ROWS, NMEL), x2d.dtype)
        padded[:MEL_ROWS] = x2d
        return padded.reshape(128, MEL_F)

    in_maps = []
    for c in range(NCORES):
        bs = slice(2 * c, 2 * c + 2)
        mp = pad_rows((mels_pred[bs] * maskf[bs][..., None])
                      .reshape(MEL_ROWS, NMEL).astype(bf))
        mt = pad_rows(mels_target[bs].reshape(MEL_ROWS, NMEL).astype(bf))

        aln = np.zeros((8, 16 * ALN_PER_PART), f8)
        core_el = el[:, 8 * c:8 * c + 8]          # [3, 8, 2975]
        aln[:, :N * ALN_PER_PLANE] = \
            core_el.transpose(1, 0, 2).reshape(8, N * ALN_PER_PLANE).astype(f8)
        aln = aln.reshape(128, ALN_PER_PART)

        dAll = np.zeros((128, W_ALL), np.uint8)
        dAll[:, O_ID:O_S2END] = ident.view(np.uint8)
        ind2 = np.zeros((128, 2), np.float32)
        ind2[:64, 0] = 1.0
        ind2[64:, 1] = 1.0
        dAll[:, O_IND:O_IND + 8] = ind2.view(np.uint8)
        dAll[:, O_ALN:O_STOP] = aln.view(np.uint8)
        st13 = np.concatenate(
            [split13(stop_pred[2 * c].astype(np.float16), np.float16(1.0)),
             split13(stop_pred[2 * c + 1].astype(np.float16), np.float16(1.0))])
        mk13 = np.concatenate(
            [split13(maskf[2 * c].astype(np.float16), np.float16(0)),
             split13(maskf[2 * c + 1].astype(np.float16), np.float16(0))])
        dAll[:, O_STOP:O_MASK] = st13.view(np.uint8)
        dAll[:, O_MASK:O_IOTA] = mk13.view(np.uint8)
        dAll[:, O_IOTA:O_IOTA + 2 * MG] = iota13.view(np.uint8)
        lenf = np.repeat(lengths[bs].astype(np.float32), 64)  # [128]
        dAll[:, O_LEN:O_LEN + 4] = lenf[:, None].view(np.uint8)
        lpre = np.zeros((128, 1), np.float32)
        lpre[0:2, 0] = lengths[bs]
        dAll[:, O_LPRE:O_LPRE + 4] = lpre.view(np.uint8)
        dAll[:, O_P1:O_T1] = mp[:, 0:MH1].view(np.uint8)
        dAll[:, O_T1:O_P2] = np.ascontiguousarray(mt[:, 0:MH1]).view(np.uint8)
        dAll[:, O_P2:O_T2] = np.ascontiguousarray(mp[:, MH1:]).view(np.uint8)
        dAll[:, O_T2:W_ALL] = np.ascontiguousarray(mt[:, MH1:]).view(np.uint8)

        in_maps.append({"dAll": dAll})
    return in_maps


def combine_partials(partials):
    """partials: list of 8 arrays [128,8] -> final scalar (0-d f32 ndarray).

    Cols 5/7 are unused (uninitialized SBUF) and ignored.
    """
    ps = np.stack([np.asarray(p, dtype=np.float64).reshape(128, 8)
                   for p in partials])
    tot = ps.sum(axis=(0, 1))
    dc_w, mask_cnt, len_sum = tot[0], tot[3], tot[4]
    melA = tot[1] + tot[6]
    sel_lnp = tot[2]
    mel_loss = melA / float(B * T * NMEL)
    stop_loss = -5.0 * sel_lnp / mask_cnt
    dc = dc_w / (H * len_sum * N)
    return np.array(np.float32(mel_loss + stop_loss - 1e-4 * dc))


def kernel(lengths, mask, stop_pred, mels_pred, mels_target, alignments):
    from concourse.bass_utils import run_bass_kernel_spmd

    nc = _get_nc()
    in_maps = make_in_maps(lengths, np.asarray(mask), stop_pred,
                           mels_pred, mels_target, alignments)
    res = run_bass_kernel_spmd(nc, in_maps, list(range(NCORES)))
    return combine_partials([r["out"] for r in res.results])


# revision 54
# speedup vs baseline: 1.0589x; 1.0047x over previous
"""Trainium2 Bass kernel for nn_Loss_20993800143146 (loss_fn).

Computes, over 8 NeuronCores (data-parallel over batch / bh):
    mel_loss  = mean(|mels_pred * mask - mels_target|)           (mean over full tensor)
    stop_loss = sum(-5 * log(stop_pred[b, last_idx_b])) / mask.sum()
    dc        = sum(alignments * band[s,t] * bmask[b]) / (H * lengths.sum() * N)
    out       = mel_loss + stop_loss - 1e-4 * dc

Key algebraic facts:
  * band[s,t] = (s >= clip(5t-50,0,160)) & (s < clip(5t+50,0,160)) is zero for
    t >= 42, and within t < 42 only 2975 of the 6720 (s,t) cells are nonzero.
    The host packs EXACTLY the banded elements densely (zero-padded to a
    rectangle), so the device just sums them - no band weights needed.
  * The mel mask multiplies mels_pred only, so it is folded into the host-side
    packing of the pred tile (masked positions packed as 0), leaving a plain
    sum(|p - t|) on device.

Sharding: batch dim (16 -> 2 per core) for mask/stop/mels, bh dim (64 -> 8 per
core) for alignments. Each core reduces its shard to a [128,8] stats tile; the
host folds partitions / cores and applies the constant-denominator arithmetic.

Heavy data in bf16 (mels) / fp8-e4m3 (alignments) / f16 (stop sidecar and
transpose identity); rel-err budget is 2e-2, measured error stays ~1e-5.

Input DMA is phased through ONE dram tensor on the sync path (the shared HW
queue set executes transfers in doorbell order at full aggregate bandwidth):
  phase S2 (first): banded alignments fp8, stop/mask/iota f16 in the
     split-per-b layout (b = p//64, t = 13*(p%64)+j), per-partition b length
     f32, length prefill f32, b-group indicator f32, f16 transpose identity -
     unblocks the stop and dc chains while the mels stream;
  phases S1a / S1b: mel parts (pred*mask | target bf16 each), split
     asymmetrically (640/400 els) - the subtract+|.|-reduce pairs pipeline
     behind the DMA and the smaller last part minimizes the post-DMA tail.

Stop-term selection per b with no serial cp->PE chain: per-partition
masked-iota max mxp is PE-transposed to a row on partition 0 via the
identity, reduced to the two per-b maxes, PE-broadcast back to all
partitions (ones-column matmul), and a single per-partition
is_equal(mxp, own-b max) * cp select writes the stats column.

Stats cols: 0=dc_w, 1=melA_h1, 2=sel_lnp, 3=mask_cnt, 4=len, 6=melA_h2;
5/7 unused. The [128,8] tile goes out raw; the host folds partitions/cores
(ignoring the unused cols) and applies the constant denominators.
"""

import numpy as np
import ml_dtypes

# Problem constants (hardcoded per contract; kernel.py must be self-contained).
H = 4
B = 16
T = 800
NMEL = 80
S = 160
N = 3
BW = 50
K = T // S  # 5
TC = 42  # band[:, t] == 0 for all t >= TC
NCORES = 8

MEL_ROWS = 2 * T            # 1600 (b,t) rows per core
MEL_PAD_ROWS = 1664         # pad to 128 * 13
MG = 13                     # 80-col groups per partition / stop t's per part
MEL_F = MG * NMEL           # 1040 mel elements per partition per tensor
MH = MEL_F // 2             # 520 els per mel half
ALN_PER_PLANE = 2975        # nonzero band cells per (n, bh) plane
ALN_PER_PART = 560          # 16*560 >= 3*2975, zero padded
ALN_HALF = ALN_PER_PART // 2  # 280

# dAll byte offsets -- phase S2
O_ALN = 0                     # 560 fp8
O_STOP = O_ALN + 560          # 560, 13 f16
O_MASK = O_STOP + 2 * MG      # 586
O_IOTA = O_MASK + 2 * MG      # 612
O_LEN = 640                   # f32
O_LPRE = 644                  # f32: lengths at partitions 0/1, 0 elsewhere
O_IND = 648                   # 2 f32 b-group indicator
O_ID = 656                    # 128 f16 identity row
O_S2END = O_ID + 256          # 912
# phases S1a / S1b -- asymmetric split: the smaller second half shortens
# the post-DMA subtract+reduce tail
MH1 = 640                     # els in mel part 1
MH2 = MEL_F - MH1             # 400 els in mel part 2
O_P1 = O_S2END                # mel pred part1, MH1 bf16
O_T1 = O_P1 + 2 * MH1         # 2192
O_P2 = O_T1 + 2 * MH1         # 3472
O_T2 = O_P2 + 2 * MH2         # 4272
W_ALL = O_T2 + 2 * MH2        # 5072

_CACHE = {}


def _band_bool():
    tr = np.arange(TC)
    mn = np.clip(K * tr - BW, 0, S)
    mx = np.clip(K * tr + BW, 0, S)
    rows = np.arange(S)
    return (rows[:, None] >= mn[None, :]) & (rows[:, None] < mx[None, :])


def _build_bass():
    import concourse.bacc as bacc
    import concourse.tile as tile
    import concourse.mybir as mybir
    from contextlib import ExitStack

    f32 = mybir.dt.float32
    f16 = mybir.dt.float16
    bf16 = mybir.dt.bfloat16
    fp8 = mybir.dt.float8e4
    u8 = mybir.dt.uint8
    Alu = mybir.AluOpType
    Act = mybir.ActivationFunctionType
    Ax = mybir.AxisListType

    nc = bacc.Bacc("TRN2", target_bir_lowering=False, debug=False,
                   num_devices=NCORES)

    dAll = nc.dram_tensor("dAll", [128, W_ALL], u8, kind="ExternalInput").ap()
    out = nc.dram_tensor("out", [128, 8], f32, kind="ExternalOutput").ap()

    with tile.TileContext(nc) as tc:
        with ExitStack() as ctx:
            pool = ctx.enter_context(tc.tile_pool(name="main", bufs=1))
            ppool = ctx.enter_context(tc.tile_pool(name="ps", bufs=1,
                                                   space="PSUM"))

            td_t = pool.tile([128, W_ALL], u8, tag="td")

            # Phased DMA triggers, all on the sync sequencer; the shared
            # queue set executes them in doorbell order: the small S2 phase
            # first (unblocks the whole stop/dc chain early), then the two
            # mel halves so the subtract+|.|-reduce pairs pipeline behind
            # the DMA.
            nc.sync.dma_start(td_t[:, 0:O_S2END], dAll[:, 0:O_S2END])
            nc.sync.dma_start(td_t[:, O_P1:O_P2], dAll[:, O_P1:O_P2])
            nc.sync.dma_start(td_t[:, O_P2:W_ALL], dAll[:, O_P2:W_ALL])

            id_v = td_t[:, O_ID:O_S2END].bitcast(f16)         # [128, 128]
            ind_v = td_t[:, O_IND:O_IND + 8].bitcast(f32)     # [128, 2]
            aln_v = td_t[:, O_ALN:O_STOP].bitcast(fp8)        # [128, 560]
            stop_v = td_t[:, O_STOP:O_MASK].bitcast(f16)      # [128, 13]
            mask_v = td_t[:, O_MASK:O_IOTA].bitcast(f16)
            iota_v = td_t[:, O_IOTA:O_IOTA + 2 * MG].bitcast(f16)
            lenf_v = td_t[:, O_LEN:O_LEN + 4].bitcast(f32)
            lpre_v = td_t[:, O_LPRE:O_LPRE + 4].bitcast(f32)  # [128, 1]
            p1_v = td_t[:, O_P1:O_T1].bitcast(bf16)           # [128, MH1]
            t1_v = td_t[:, O_T1:O_P2].bitcast(bf16)
            p2_v = td_t[:, O_P2:O_T2].bitcast(bf16)
            t2_v = td_t[:, O_T2:W_ALL].bitcast(bf16)

            st_t = pool.tile([128, 8], f32, tag="st")
            stats = st_t[:]
            ones_t = pool.tile([1, 128], f16, tag="ones")
            nc.vector.memset(ones_t[:], 1.0)

            # ---- ACT: Ln for the stop term ----
            lnp_t = pool.tile([128, MG], f32, tag="lnp")
            nc.scalar.activation(lnp_t[:], stop_v, Act.Ln)

            # ---- DVE stop front + dc term (phase S2 data only) ----
            tl_t = pool.tile([128, MG], f32, tag="tl")
            nc.vector.scalar_tensor_tensor(
                tl_t[:], iota_v, 1.0, mask_v, op0=Alu.bypass, op1=Alu.mult)
            mxp_t = pool.tile([128, 1], f32, tag="mxp")
            nc.vector.tensor_reduce(mxp_t[:], tl_t[:], axis=Ax.X, op=Alu.max)
            nc.vector.tensor_reduce(stats[:, 3:4], mask_v, axis=Ax.X,
                                    op=Alu.add)
            nc.vector.tensor_copy(stats[:, 4:5], lpre_v)
            asum_t = pool.tile([128, ALN_HALF], bf16, tag="asum")
            dcs_t = pool.tile([128, 1], f32, tag="dcs")
            nc.vector.scalar_tensor_tensor(
                asum_t[:], aln_v[:, 0:ALN_HALF], 1.0, aln_v[:, ALN_HALF:],
                op0=Alu.bypass, op1=Alu.add, accum_out=dcs_t[:])
            nc.vector.scalar_tensor_tensor(
                stats[:, 0:1], lenf_v, float(T), dcs_t[:],
                op0=Alu.is_le, op1=Alu.mult)
            eq_t = pool.tile([128, MG], f32, tag="eq")
            cp_t = pool.tile([128, 1], f32, tag="cp")
            nc.vector.scalar_tensor_tensor(
                eq_t[:], tl_t[:], mxp_t[:, 0:1], lnp_t[:],
                op0=Alu.is_equal, op1=Alu.mult, accum_out=cp_t[:])

            # ---- stop tail: transpose mxp, per-b max, broadcast back,
            # then a per-partition select (no serial cp->PE chain) ----
            mxp16_t = pool.tile([128, 1], f16, tag="mxp16")
            nc.vector.tensor_copy(mxp16_t[:], mxp_t[:])
            psA = ppool.tile([1, 128], f32, tag="psA")
            nc.tensor.matmul(psA[:], lhsT=mxp16_t[:], rhs=id_v,
                             start=True, stop=True)
            mb_t = pool.tile([1, 2], f16, tag="mb")
            nc.vector.tensor_reduce(
                mb_t[:], psA[:].rearrange("p (b g) -> p b g", g=64),
                axis=Ax.X, op=Alu.max)
            bc2 = ppool.tile([128, 2], f32, tag="bc2")
            nc.tensor.matmul(bc2[:], lhsT=ones_t[:], rhs=mb_t[0:1, :],
                             start=True, stop=True)
            gs2_t = pool.tile([128, 2], f32, tag="gs2")
            gmx_t = pool.tile([128, 1], f32, tag="gmx")
            nc.vector.scalar_tensor_tensor(
                gs2_t[:], bc2[:], 1.0, ind_v,
                op0=Alu.bypass, op1=Alu.mult, accum_out=gmx_t[:])
            nc.vector.scalar_tensor_tensor(
                stats[:, 2:3], mxp_t[:], gmx_t[:, 0:1], cp_t[:],
                op0=Alu.is_equal, op1=Alu.mult)

            # ---- mel term: halves pipelined behind phases S1a / S1b ----
            d_t = pool.tile([128, MEL_F], bf16, tag="d")
            nc.vector.tensor_sub(d_t[:, 0:MH1], p1_v, t1_v)
            nc.vector.tensor_reduce(stats[:, 1:2], d_t[:, 0:MH1], axis=Ax.X,
                                    op=Alu.add, apply_absolute_value=True)
            nc.vector.tensor_sub(d_t[:, MH1:MEL_F], p2_v, t2_v)
            nc.vector.tensor_reduce(stats[:, 6:7], d_t[:, MH1:MEL_F],
                                    axis=Ax.X, op=Alu.add,
                                    apply_absolute_value=True)

            # ---- stats go out raw; the host folds the 128 partitions ----
            nc.sync.dma_start(out, st_t[:])

    nc.compile()
    return nc


def _get_nc():
    if "nc" not in _CACHE:
        _CACHE["nc"] = _build_bass()
    return _CACHE["nc"]


def make_in_maps(lengths, mask, stop_pred, mels_pred, mels_target, alignments):
    """Shard full inputs into the 8 per-core input dicts."""
    lengths = np.ascontiguousarray(lengths, dtype=np.int32)
    maskf = np.ascontiguousarray(mask).astype(np.float32)
    stop_pred = np.ascontiguousarray(stop_pred, dtype=np.float32)
    mels_pred = np.ascontiguousarray(mels_pred, dtype=np.float32)
    mels_target = np.ascontiguousarray(mels_target, dtype=np.float32)
    alignments = np.ascontiguousarray(alignments, dtype=np.float32)

    bf = ml_dtypes.bfloat16
    f8 = ml_dtypes.float8_e4m3
    band = _band_bool()  # [S, TC]
    el = alignments[:, :, :, :TC][:, :, band]  # [N, B*H, 2975]

    def split13(row, pad):
        o = np.full((64 * MG,), pad, row.dtype)
        o[:T] = row
        return o.reshape(64, MG)

    iota13 = np.concatenate([split13(np.arange(1, T + 1, dtype=np.float16),
                                     np.float16(0))] * 2)  # [128, 13]
    ident = np.eye(128, dtype=np.float16)

    def pad_rows(x2d):
        padded = np.zeros((MEL_PAD_ROWS, NMEL), x2d.dtype)
        padded[:MEL_ROWS] = x2d
        return padded.reshape(128, MEL_F)

    in_maps = []
    for c in range(NCORES):
        bs = slice(2 * c, 2 * c + 2)
        mp = pad_rows((mels_pred[bs] * maskf[bs][..., None])
                      .reshape(MEL_ROWS, NMEL).astype(bf))
        mt = pad_rows(mels_target[bs].reshape(MEL_ROWS, NMEL).astype(bf))

        aln = np.zeros((8, 16 * ALN_PER_PART), f8)
        core_el = el[:, 8 * c:8 * c + 8]          # [3, 8, 2975]
        aln[:, :N * ALN_PER_PLANE] = \
            core_el.transpose(1, 0, 2).reshape(8, N * ALN_PER_PLANE).astype(f8)
        aln = aln.reshape(128, ALN_PER_PART)

        dAll = np.zeros((128, W_ALL), np.uint8)
        dAll[:, O_ID:O_S2END] = ident.view(np.uint8)
        ind2 = np.zeros((128, 2), np.float32)
        ind2[:64, 0] = 1.0
        ind2[64:, 1] = 1.0
        dAll[:, O_IND:O_IND + 8] = ind2.view(np.uint8)
        dAll[:, O_ALN:O_STOP] = aln.view(np.uint8)
        st13 = np.concatenate(
            [split13(stop_pred[2 * c].astype(np.float16), np.float16(1.0)),
             split13(stop_pred[2 * c + 1].astype(np.float16), np.float16(1.0))])
        mk13 = np.concatenate(
            [split13(maskf[2 * c].astype(np.float16), np.float16(0)),
             split13(maskf[2 * c + 1].astype(np.float16), np.float16(0))])
        dAll[:, O_STOP:O_MASK] = st13.view(np.uint8)
        dAll[:, O_MASK:O_IOTA] = mk13.view(np.uint8)
        dAll[:, O_IOTA:O_IOTA + 2 * MG] = iota13.view(np.uint8)
        lenf = np.repeat(lengths[bs].astype(np.float32), 64)  # [128]
        dAll[:, O_LEN:O_LEN + 4] = lenf[:, None].view(np.uint8)
        lpre = np.zeros((128, 1), np.float32)
        lpre[0:2, 0] = lengths[bs]
        dAll[:, O_LPRE:O_LPRE + 4] = lpre.view(np.uint8)
        dAll[:, O_P1:O_T1] = mp[:, 0:MH1].view(np.uint8)
        dAll[:, O_T1:O_P2] = np.ascontiguousarray(mt[:, 0:MH1]).view(np.uint8)
        dAll[:, O_P2:O_T2] = np.ascontiguousarray(mp[:, MH1:]).view(np.uint8)
        dAll[:, O_T2:W_ALL] = np.ascontiguousarray(mt[:, MH1:]).view(np.uint8)

        in_maps.append({"dAll": dAll})
    return in_maps


def combine_partials(partials):
    """partials: list of 8 arrays [128,8] -> final scalar (0-d f32 ndarray).

    Cols 5/7 are unused (uninitialized SBUF) and ignored.
    """
    ps = np.stack([np.asarray(p, dtype=np.float64).reshape(128, 8)
                   for p in partials])
    tot = ps.sum(axis=(0, 1))
    dc_w, mask_cnt, len_sum = tot[0], tot[3], tot[4]
    melA = tot[1] + tot[6]
    sel_lnp = tot[2]
    mel_loss = melA / float(B * T * NMEL)
    stop_loss = -5.0 * sel_lnp / mask_cnt
    dc = dc_w / (H * len_sum * N)
    return np.array(np.float32(mel_loss + stop_loss - 1e-4 * dc))


def kernel(lengths, mask, stop_pred, mels_pred, mels_target, alignments):
    from concourse.bass_utils import run_bass_kernel_spmd

    nc = _get_nc()
    in_maps = make_in_maps(lengths, np.asarray(mask), stop_pred,
                           mels_pred, mels_target, alignments)
    res = run_bass_kernel_spmd(nc, in_maps, list(range(NCORES)))
    return combine_partials([r["out"] for r in res.results])
